# revision 13
# baseline (speedup 1.0000x reference)
"""Trainium2 Bass kernel for nn_Matching_layer (9x9 local correlation volume).

Computation (per batch element b):
    f1n = l2normalize(feature1[b]) over C;  f2n = l2normalize(feature2[b])
    out[b, dh*9+dw, y*64+x] = relu(<f2n[:, y+dh-4, x+dw-4], f1n[:, y, x]>)
    (out-of-range f2 positions contribute exactly 0)

Shapes: feature1/2 (16, 512, 64, 64) fp32 -> out (16, 81, 4096) fp32.

Strategy (8 NeuronCores, pure data parallelism, 2 images per core):
  * bf16 everywhere on-chip (cast during the input DMA).
  * f2 kept resident in SBUF as a y-padded plane [128c x 4chunk x (72*64+8)];
    x-borders are handled by masking the affected outputs at the end.
  * f2 is normalized in-place (ACT squares -> PE ones-reduce -> sqrt ->
    reciprocal -> K=1 broadcast matmul -> DVE multiply).
  * f1 stays unnormalized; rn1 is computed per 16x8 tile directly in
    tile-partition order ([128,1] per tile) and applied after extraction.
  * Main compute: per 16x8 position tile, PE computes the banded Gram tile
    G[128 pos, 384 window-pos] = f1_tile^T @ f2_window (4 K-chunks of 128).
  * The 81 window dot products per position sit on a per-partition diagonal
    of G, which no on-chip engine can address.  G is therefore relu'd,
    cast to bf16, and DMA'd to a DRAM scratch buffer in a *padded* layout
    addr = py*3128 + px*392 + q  chosen so that the needed elements sit at
    addr = p*393 + dh*16 + dw  -- an affine access pattern.  A single
    gather DMA per tile brings back [128, 81].
  * Scale by rn1, mask x-borders, PE-transpose to [81, 128], accumulate the
    [81, 4096] output image in SBUF, one DMA out per image.
"""

import threading

import numpy as np

import concourse.bass as bass
import concourse.mybir as mybir
import concourse.tile as tile
from concourse.masks import make_identity
from concourse.vector_clock import ScopedClock

# ---------------------------------------------------------------- constants
B, C, H, W = 16, 512, 64, 64
PATCH, R = 9, 4
P2 = PATCH * PATCH            # 81
HWTOT = H * W                 # 4096
N_CORES = 8
B_LOC = B // N_CORES          # 2 images per core
NCH = C // 128                # 4 contraction chunks

BY, BX = 16, 8                # position tile (M = 128)
NTY, NTX = H // BY, W // BX   # 4 x 8 = 32 tiles per image
QY, QX = BY + 2 * R, BX + 2 * R   # 24 x 16 window block
Q = QY * QX                   # 384

# f2 plane: y-padded (R rows top/bottom), x handled by masks; 4-elem guards
PF = (H + 2 * R) * W + 2 * R          # 72*64 + 8 = 4616
PORIGIN = R                           # flat offset of plane (y=-4, x=0)
PINT = PORIGIN + R * W                # interior start = 4 + 256 = 260

# skewed DRAM layout for G:  addr = py*S_PY + px*S_PX + q
# needed element (p, dh, dw) then sits at  p*PSTEP + dh*QX + dw
S_PX = QX * (QX - 1) + Q // QX + 0    # see derivation: S_PX = 392
S_PX = 392
S_PY = BX * (S_PX + 1) - QX           # 8*393 - 16 = 3128
PSTEP = S_PX + 1                      # 393
GSIZE = (BY - 1) * S_PY + (BX - 1) * S_PX + Q   # 50048

FP32 = mybir.dt.float32
BF16 = mybir.dt.bfloat16
AFT = mybir.ActivationFunctionType


# -------------------------------------------------- tile tail-drain workaround
# The walrus build in this container rejects a Drain instruction carrying more
# than one sync wait.  Split the tail waits into single-wait NOPs instead.
def _patched_drain_and_barrier(self, tick_clock, wait_clock):
    nc = self.nc
    probe = nc.sync.nop(nofuse=True)
    wait_clock.add_sem_waits(probe.ins, ScopedClock({None: tick_clock.global_clock}))
    waits = list(probe.ins.sync_info.on_wait)
    if len(waits) > 1:
        probe.ins.sync_info.on_wait = waits[:1]
        id2sem = {s.num: s for s in self.sems.allocated().values()}
        for w in waits[1:]:
            extra = nc.sync.nop(nofuse=True)
            extra.wait_op(id2sem[w.id], w.wait_value, "sem-ge")
    nc.sync.drain()
    nc.all_engine_barrier()
    popped = nc._tile_sem_poison_stack.pop()
    assert popped is self._sem_poison
    nc.clear_and_free_semaphores(list(self.sems.allocated().values()))
    nc.all_engine_barrier()


tile.TileContext._drain_and_barrier = _patched_drain_and_barrier


def _split_sync_waits(nc, max_waits=1):
    """The walrus build here only supports a limited number of sync waits per
    instruction.  Move excess waits onto engine-matched NOPs inserted just
    before the owning instruction (semantics preserved: the engine blocks on
    the nops first)."""
    import copy as _copy

    tmpl = None
    for f in nc.m.functions:
        for bb in f.blocks:
            for inst in bb.instructions:
                if inst.opcode == "NoOp":
                    tmpl = inst
                    break
            if tmpl is not None:
                break
        if tmpl is not None:
            break
    assert tmpl is not None, "no NoOp template found"
    uid = 0
    for f in nc.m.functions:
        for bb in f.blocks:
            new = []
            changed = False
            for inst in bb.instructions:
                si = inst.sync_info
                if si is not None and len(si.on_wait) > max_waits:
                    waits = list(si.on_wait)
                    extra, keep = waits[:-max_waits], waits[-max_waits:]
                    for i in range(0, len(extra), max_waits):
                        nop = _copy.deepcopy(tmpl)
                        nop.name = f"I-waitsplit-{uid}"
                        uid += 1
                        nop.engine = inst.engine
                        nop.sync_info = mybir.SyncInfo(
                            on_wait=extra[i : i + max_waits], on_update=[]
                        )
                        new.append(nop)
                    si.on_wait = keep
                    changed = True
                new.append(inst)
            if changed:
                bb.instructions = new


def _sub_ap(t, extra_offset, dims):
    """AP on t's tensor at t.offset + extra_offset with partition dim kept."""
    return bass.AP(
        tensor=t.tensor, offset=t.offset + extra_offset, ap=[list(t.ap[0])] + dims
    )


def _flat_ap(t, extra_offset, dims):
    """AP on a DRAM tile viewed as flat memory (no partition dim)."""
    return bass.AP(tensor=t.tensor, offset=t.offset + extra_offset, ap=dims)


def build_matching_kernel(nc, f1, f2, mask0, mask7, out):
    """Emit Tile IR.  f1/f2: [B_LOC, C, H, W] fp32 DRAM; masks: [128, P2] fp32;
    out: [B_LOC, P2, H*W] fp32 DRAM."""
    from contextlib import ExitStack

    with tile.TileContext(nc) as tc, ExitStack() as ctx:
        consts = ctx.enter_context(tc.tile_pool(name="consts", bufs=1))
        planes = ctx.enter_context(tc.tile_pool(name="planes", bufs=2))
        f1pool = ctx.enter_context(tc.tile_pool(name="f1pool", bufs=2))
        sqpool = ctx.enter_context(tc.tile_pool(name="sqpool", bufs=3))
        strip = ctx.enter_context(tc.tile_pool(name="strip", bufs=2))
        rn1pool = ctx.enter_context(tc.tile_pool(name="rn1", bufs=2))
        gsb_pool = ctx.enter_context(tc.tile_pool(name="gsb", bufs=3))
        exb_pool = ctx.enter_context(tc.tile_pool(name="exb", bufs=4))
        exf_pool = ctx.enter_context(tc.tile_pool(name="exf", bufs=NTY * NTX))
        outpool = ctx.enter_context(tc.tile_pool(name="outimg", bufs=1))

        ps_ssq = ctx.enter_context(tc.tile_pool(name="ps_ssq", bufs=1, space="PSUM"))
        ps_bc = ctx.enter_context(tc.tile_pool(name="ps_bc", bufs=2, space="PSUM"))
        ps_g = ctx.enter_context(tc.tile_pool(name="ps_g", bufs=2, space="PSUM"))
        ps_tp = ctx.enter_context(tc.tile_pool(name="ps_tp", bufs=2, space="PSUM"))
        ps_rn1 = ctx.enter_context(tc.tile_pool(name="ps_rn1", bufs=1, space="PSUM"))

        gdram = ctx.enter_context(tc.tile_pool(name="gdram", bufs=6, space="DRAM"))

        # ---------------- constants
        ident = consts.tile([128, 128], FP32)
        make_identity(nc, ident)
        ident1 = consts.tile([1, 1], FP32)
        nc.vector.memset(ident1, 1.0)
        ones_col = consts.tile([128, 1], BF16)
        nc.vector.memset(ones_col, 1.0)
        ones_row = consts.tile([1, 128], BF16)
        nc.vector.memset(ones_row, 1.0)
        m0 = consts.tile([128, P2], FP32)
        nc.sync.dma_start(out=m0, in_=mask0[:, :])
        m7 = consts.tile([128, P2], FP32)
        nc.sync.dma_start(out=m7, in_=mask7[:, :])
        eps = consts.tile([1, 1], FP32)
        nc.vector.memset(eps, 1e-6)

        plane = []
        f1p = []
        # ---------------- load both images up front (planes are double-buffered)
        for img in range(B_LOC):
            pl = planes.tile([128, NCH, PF], BF16)
            fp = f1pool.tile([128, NCH, HWTOT], BF16)
            plane.append(pl)
            f1p.append(fp)
            for kc in range(NCH):
                nc.vector.memset(pl[:, kc, 0:PINT], 0.0)
                nc.vector.memset(pl[:, kc, PINT + HWTOT : PF], 0.0)
                nc.gpsimd.dma_start(
                    out=pl[:, kc, PINT : PINT + HWTOT],
                    in_=f2[img, kc * 128 : (kc + 1) * 128, :, :],
                )
                nc.gpsimd.dma_start(
                    out=fp[:, kc, :], in_=f1[img, kc * 128 : (kc + 1) * 128, :, :]
                )

        rn1s = []
        # ---------------- normalization for both images
        for img in range(B_LOC):
            pl, fp = plane[img], f1p[img]
            # --- f2: strip-wise (8 strips of 512 positions), normalize in-place
            for s in range(8):
                off = PINT + 512 * s
                sq4 = sqpool.tile([128, NCH, 512], BF16, tag="sq4")
                nc.scalar.activation(
                    out=sq4, in_=pl[:, :, off : off + 512], func=AFT.Square
                )
                ssq = ps_ssq.tile([1, 512], FP32)
                for kc in range(NCH):
                    nc.tensor.matmul(
                        ssq,
                        lhsT=ones_col,
                        rhs=sq4[:, kc, :],
                        start=(kc == 0),
                        stop=(kc == NCH - 1),
                    )
                std = strip.tile([1, 512], FP32, tag="std")
                nc.scalar.activation(out=std, in_=ssq, func=AFT.Sqrt, bias=eps)
                rnf = strip.tile([1, 512], FP32, tag="rnf")
                nc.vector.reciprocal(rnf, std)
                rnb = strip.tile([1, 512], BF16, tag="rnb")
                nc.vector.tensor_copy(out=rnb, in_=rnf)
                bc = ps_bc.tile([128, 512], FP32)
                nc.tensor.matmul(bc, lhsT=ones_row, rhs=rnb, start=True, stop=True)
                for kc in range(NCH):
                    seg = pl[:, kc, off : off + 512]
                    nc.vector.tensor_mul(seg, seg, bc)
            # --- f1: per-tile, rn1 lands in tile-partition order [128, NT]
            rs = rn1pool.tile([128, NTY * NTX], FP32)
            rn1s.append(rs)
            f1v = [
                fp[:, kc, :].rearrange("p (y x) -> p y x", x=W) for kc in range(NCH)
            ]
            for ty in range(NTY):
                for tx in range(NTX):
                    t = ty * NTX + tx
                    sqT = sqpool.tile([128, NCH, BY, BX], BF16, tag="sqT")
                    nc.scalar.activation(
                        out=sqT,
                        in_=fp.rearrange("p k (y x) -> p k y x", x=W)[
                            :, :, ty * BY : (ty + 1) * BY, tx * BX : (tx + 1) * BX
                        ],
                        func=AFT.Square,
                    )
                    ssqT = ps_ssq.tile([1, 128], FP32, tag="ssq")
                    for kc in range(NCH):
                        nc.tensor.matmul(
                            ssqT,
                            lhsT=ones_col,
                            rhs=sqT[:, kc, :, :],
                            start=(kc == 0),
                            stop=(kc == NCH - 1),
                        )
                    stdT = strip.tile([1, 128], FP32, tag="stdT")
                    nc.scalar.activation(out=stdT, in_=ssqT, func=AFT.Sqrt, bias=eps)
                    rnTp = ps_rn1.tile([128, 1], FP32)
                    nc.tensor.transpose(rnTp, stdT, ident1)
                    nc.vector.reciprocal(rs[:, t : t + 1], rnTp)

        # ---------------- main per-tile compute
        for img in range(B_LOC):
            pl, fp, rs = plane[img], f1p[img], rn1s[img]
            oimg = outpool.tile([81, HWTOT], FP32)
            exfs = []
            fpv = fp.rearrange("p k (y x) -> p k y x", x=W)
            for ty in range(NTY):
                for tx in range(NTX):
                    t = ty * NTX + tx
                    # stationary operand must be a single-free-dim AP: stage the
                    # strided 16x8 f1 tile to a contiguous buffer on ACT
                    f1s = sqpool.tile([128, NCH, BY, BX], BF16, tag="f1s")
                    nc.scalar.copy(
                        out=f1s,
                        in_=fpv[
                            :, :, ty * BY : (ty + 1) * BY, tx * BX : (tx + 1) * BX
                        ],
                    )
                    f1sm = f1s.rearrange("p k y x -> p k (y x)")
                    gps = ps_g.tile([128, Q], FP32)
                    for kc in range(NCH):
                        rhs = _sub_ap(
                            pl[:, kc, :],
                            ty * BY * W + tx * BX,
                            [[W, QY], [1, QX]],
                        )
                        nc.tensor.matmul(
                            gps, lhsT=f1sm[:, kc, :], rhs=rhs,
                            start=(kc == 0), stop=(kc == NCH - 1),
                        )
                    gsb = gsb_pool.tile([128, Q], BF16)
                    nc.vector.tensor_scalar_max(gsb, gps, 0.0)
                    gd = gdram.tile([1, GSIZE], BF16)
                    nc.sync.dma_start(
                        out=_flat_ap(gd, 0, [[S_PY, BY], [S_PX, BX], [1, Q]]),
                        in_=gsb,
                    )
                    exb = exb_pool.tile([128, P2], BF16)
                    nc.gpsimd.dma_start(
                        out=exb.rearrange("p (a b) -> p a b", b=PATCH),
                        in_=_flat_ap(
                            gd, 0, [[PSTEP, 128], [QX, PATCH], [1, PATCH]]
                        ),
                    )
                    exf = exf_pool.tile([128, P2], FP32)
                    nc.vector.tensor_scalar_mul(exf, exb, rs[:, t : t + 1])
                    if tx == 0:
                        nc.vector.tensor_mul(exf, exf, m0)
                    elif tx == NTX - 1:
                        nc.vector.tensor_mul(exf, exf, m7)
                    exfs.append((ty, tx, exf))
            # transposes batched after the tile loop so PE never stalls on the
            # G -> DRAM -> gather roundtrip of the same tile
            ov = oimg[:, :].rearrange("p (y x) -> p y x", x=W)
            for ty, tx, exf in exfs:
                tp = ps_tp.tile([81, 128], FP32)
                nc.tensor.transpose(tp, exf, ident)
                nc.vector.tensor_copy(
                    out=ov[:, ty * BY : (ty + 1) * BY, tx * BX : (tx + 1) * BX],
                    in_=tp.rearrange("p (a b) -> p a b", b=BX),
                )
            nc.sync.dma_start(out=out[img, :, :], in_=oimg)
    return nc


# ---------------------------------------------------------------- host side
def _edge_masks():
    p = np.arange(128)
    d = np.arange(P2)
    px = (p % BX)[:, None]
    dw = (d % PATCH)[None, :]
    # tx = 0:      x_img = px + dw - R >= 0             <=>  px + dw >= R
    # tx = NTX-1:  x_img = (NTX-1)*BX + px + dw - R < W <=>  px + dw < BX + R
    m0 = (px + dw >= R).astype(np.float32)
    m7 = (px + dw < BX + R).astype(np.float32)
    return m0, m7


_cache = threading.local()


def _get_compiled():
    if getattr(_cache, "nc", None) is None:
        nc = bass.Bass()
        f1 = nc.dram_tensor("feature1", [B_LOC, C, H, W], FP32, kind="ExternalInput")
        f2 = nc.dram_tensor("feature2", [B_LOC, C, H, W], FP32, kind="ExternalInput")
        mask0 = nc.dram_tensor("mask0", [128, P2], FP32, kind="ExternalInput")
        mask7 = nc.dram_tensor("mask7", [128, P2], FP32, kind="ExternalInput")
        out = nc.dram_tensor("out", [B_LOC, P2, HWTOT], FP32, kind="ExternalOutput")
        build_matching_kernel(nc, f1.ap(), f2.ap(), mask0.ap(), mask7.ap(), out.ap())
        _split_sync_waits(nc, max_waits=1)
        _cache.nc = nc
    return _cache.nc


def kernel(feature1: np.ndarray, feature2: np.ndarray) -> np.ndarray:
    from concourse.bass_utils import run_bass_kernel_spmd

    feature1 = np.ascontiguousarray(feature1, dtype=np.float32)
    feature2 = np.ascontiguousarray(feature2, dtype=np.float32)
    nc = _get_compiled()
    m0, m7 = _edge_masks()
    in_maps = []
    for c in range(N_CORES):
        sl = slice(c * B_LOC, (c + 1) * B_LOC)
        in_maps.append(
            {
                "feature1": feature1[sl],
                "feature2": feature2[sl],
                "mask0": m0,
                "mask7": m7,
            }
        )
    res = run_bass_kernel_spmd(nc, in_maps, core_ids=list(range(N_CORES)))
    out = np.concatenate([res.results[c]["out"] for c in range(N_CORES)], axis=0)
    return out.reshape(B, P2, HWTOT)
